# revision 4
# baseline (speedup 1.0000x reference)
"""LocalClipLoss Trainium2 kernel (8-core SPMD, caption-sharded).

Self-contained: hardcodes shapes B=32, S=196, C=768, W=97, 8 cores x 4 captions.

Per core (captions caps = [4c..4c+3], images permuted so the core's own 4
images land at iterations 0..3):
  loop over 32 images b:
    scores[s,q]  = img_b @ words  (+mask bias via appended contraction row)
    E1 = exp(scores - rowmax), rowsum (fused ACT accum)     [softmax over q]
    E2 = exp(temp1 * E1 / rowsum)                           [unnormalized]
    G = img_b @ img_b^T (Gram);  GE2 = G @ E2
    num_u[q]  = sum_s E2*scores   (ones-matmul of E2 .* scores)
    wsq_u[q]  = sum_s E2*GE2      ( = ||wei_u[q]||^2 via Gram identity )
    row_sim   = num_u / sqrt(wnorm^2 * wsq_u)   (colsum cancels; EPS clamp
                is a provable no-op for this data regime)
    simexp[:, b] = exp(temp2 * row_sim)
    at iterations 0..3: att_maps[n] = (E2 / colsum_s E2)^T * mask
  per caption: lsum = mask^T @ simexp  -> out_sims = log(lsum)
Host: similarities = gathered logs * temp3 -> symmetric CE in float64.
"""
import os
import sys
from contextlib import ExitStack

import numpy as np

sys.path.insert(0, "/opt/trn_rl_repo")
sys.path.insert(0, "/root/.axon_site/_ro/trn_rl_repo")

import concourse.bacc as bacc
import concourse.tile as tile
from concourse import mybir
from concourse.bass_utils import run_bass_kernel_spmd

FP = mybir.dt.float32
AF = mybir.ActivationFunctionType
OP = mybir.AluOpType
AX = mybir.AxisListType

B, S, C, W = 32, 196, 768, 97
NCORE, NLOC = 8, 4
W4 = NLOC * W          # 388
KCH = C // 128         # 6
PCH = [(0, 128), (128, 68)]   # S = 196 partition chunks
NEG = -30000.0

_NC_CACHE = {}
LAST_RESULTS = None


def _build():
    nc = bacc.Bacc()
    d_imgt = nc.declare_dram_parameter("imgt", [B, C, S], FP, isOutput=False)
    d_words = nc.declare_dram_parameter("words", [C, W4], FP, isOutput=False)
    d_bias = nc.declare_dram_parameter("biasrow", [1, W4], FP, isOutput=False)
    d_wordt = nc.declare_dram_parameter("wordt", [W, NLOC * C], FP, isOutput=False)
    d_maskt = nc.declare_dram_parameter("maskt", [W, NLOC], FP, isOutput=False)
    d_temps = nc.declare_dram_parameter("temps", [128, 2], FP, isOutput=False)
    d_eye = nc.declare_dram_parameter("eye", [128, 128], FP, isOutput=False)
    d_sims = nc.declare_dram_parameter("out_sims", [B, NLOC], FP, isOutput=True)
    d_att = nc.declare_dram_parameter("out_att", [NLOC, W, S], FP, isOutput=True)

    with tile.TileContext(nc) as tc, ExitStack() as ctx:
        pers = ctx.enter_context(tc.tile_pool(name="pers", bufs=1))
        setup = ctx.enter_context(tc.tile_pool(name="setup", bufs=1))
        p_img = ctx.enter_context(tc.tile_pool(name="p_img", bufs=2))
        p_e = ctx.enter_context(tc.tile_pool(name="p_e", bufs=3))
        p_gsb = ctx.enter_context(tc.tile_pool(name="p_gsb", bufs=2))
        p_v = ctx.enter_context(tc.tile_pool(name="p_v", bufs=3))
        p_att = ctx.enter_context(tc.tile_pool(name="p_att", bufs=2))
        # PSUM: scores 2tags x bufs2 = 4 banks, gram/ge2/attT shared 3 = 3, nw 1
        ps_sc = ctx.enter_context(tc.tile_pool(name="ps_sc", bufs=2, space="PSUM"))
        ps_g = ctx.enter_context(tc.tile_pool(name="ps_g", bufs=3, space="PSUM"))
        ps_nw = ctx.enter_context(tc.tile_pool(name="ps_nw", bufs=1, space="PSUM"))

        # ---- setup: persistent SBUF ----
        words_sb = pers.tile([128, KCH * W4], FP)
        nc.sync.dma_start(words_sb[:].rearrange("p (k n) -> p k n", k=KCH),
                          d_words[:].rearrange("(k p) n -> p k n", k=KCH))
        bias_sb = pers.tile([1, W4], FP)
        nc.sync.dma_start(bias_sb[:], d_bias[:])
        maskt_sb = pers.tile([W, NLOC], FP)
        nc.sync.dma_start(maskt_sb[:], d_maskt[:])
        temps_sb = pers.tile([128, 2], FP)
        nc.sync.dma_start(temps_sb[:], d_temps[:])
        eye_sb = pers.tile([128, 128], FP)
        nc.sync.dma_start(eye_sb[:], d_eye[:])
        onescol = pers.tile([128, 1], FP)
        nc.vector.memset(onescol[:], 1.0)
        onesrow = pers.tile([1, S], FP)
        nc.vector.memset(onesrow[:], 1.0)

        # wnorm^2 per (q, n) from wordt
        wordt_sb = setup.tile([W, NLOC * C], FP)
        nc.sync.dma_start(wordt_sb[:], d_wordt[:])
        wsq_scr = setup.tile([W, C], FP)
        wnormsq = pers.tile([W, NLOC], FP)
        for n in range(NLOC):
            nc.scalar.activation(wsq_scr[:], wordt_sb[:, n * C:(n + 1) * C],
                                 AF.Square, accum_out=wnormsq[:, n:n + 1])

        simexp_all = pers.tile([W, B * NLOC], FP)   # [q, (b, n)]
        se_view = simexp_all[:].rearrange("q (b n) -> q b n", n=NLOC)

        for b in range(B):
            with nc.named_scope(f"iter{b}"):
                it = p_img.tile([128, KCH * S], FP)
                itv = it[:].rearrange("p (k s) -> p k s", k=KCH)
                src = d_imgt[:][b].rearrange("(k p) s -> p k s", k=KCH)
                nc.sync.dma_start(itv[:, 0:3, :], src[:, 0:3, :])
                nc.gpsimd.dma_start(itv[:, 3:KCH, :], src[:, 3:KCH, :])

                # ---- scores + gram matmuls ----
                s_ps, g_ps = [], []
                for mi, (mo, mw) in enumerate(PCH):
                    sp = ps_sc.tile([mw, W4], FP, tag=f"sc{mi}")
                    for k in range(KCH):
                        nc.tensor.matmul(sp[:], lhsT=itv[:, k, mo:mo + mw],
                                         rhs=words_sb[:].rearrange(
                                             "p (k n) -> p k n", k=KCH)[:, k, :],
                                         start=(k == 0), stop=False)
                    nc.tensor.matmul(sp[:], lhsT=onesrow[:, 0:mw], rhs=bias_sb[:],
                                     start=False, stop=True)
                    s_ps.append(sp)
                    gp = ps_g.tile([mw, S], FP, tag="g")
                    for k in range(KCH):
                        nc.tensor.matmul(gp[:], lhsT=itv[:, k, mo:mo + mw],
                                         rhs=itv[:, k, :],
                                         start=(k == 0), stop=(k == KCH - 1))
                    g_ps.append(gp)

                # gram PSUM -> SBUF (lhsT for GE2 must be SBUF)
                g_sb = []
                for mi, (mo, mw) in enumerate(PCH):
                    gs = p_gsb.tile([mw, S], FP, tag=f"gsb{mi}")
                    nc.vector.tensor_copy(gs[:], g_ps[mi][:])
                    g_sb.append(gs)

                # ---- softmax over words (per s-chunk) ----
                negm, e1, rowsum = [], [], []
                for mi, (mo, mw) in enumerate(PCH):
                    nm = p_v.tile([mw, NLOC], FP, tag=f"nm{mi}")
                    nc.vector.tensor_reduce(
                        nm[:], s_ps[mi][:].rearrange("p (n w) -> p n w", n=NLOC),
                        axis=AX.X, op=OP.max, negate=True)
                    negm.append(nm)
                    e = p_e.tile([mw, W4], FP, tag=f"e1{mi}")
                    rs = p_v.tile([mw, NLOC], FP, tag=f"rs{mi}")
                    for n in range(NLOC):
                        nc.scalar.activation(
                            e[:, n * W:(n + 1) * W], s_ps[mi][:, n * W:(n + 1) * W],
                            AF.Exp, bias=nm[:, n:n + 1],
                            accum_out=rs[:, n:n + 1])
                    e1.append(e)
                    rowsum.append(rs)

                # t1r = temp1 / rowsum ; E2 = exp(E1 * t1r)
                e2 = []
                for mi, (mo, mw) in enumerate(PCH):
                    rc = p_v.tile([mw, NLOC], FP, tag=f"rc{mi}")
                    nc.vector.reciprocal(rc[:], rowsum[mi][:])
                    t1r = p_v.tile([mw, NLOC], FP, tag=f"t1r{mi}")
                    nc.vector.tensor_scalar_mul(t1r[:], rc[:], temps_sb[0:mw, 0:1])
                    e = p_e.tile([mw, W4], FP, tag=f"e2{mi}")
                    for n in range(NLOC):
                        nc.scalar.activation(
                            e[:, n * W:(n + 1) * W], e1[mi][:, n * W:(n + 1) * W],
                            AF.Exp, scale=t1r[:, n:n + 1])
                    e2.append(e)

                # ---- GE2 = G @ E2 ----
                ge_ps = []
                for mi, (mo, mw) in enumerate(PCH):
                    gep = ps_g.tile([mw, W4], FP, tag="g")
                    for ki, (ko, kw) in enumerate(PCH):
                        nc.tensor.matmul(gep[:], lhsT=g_sb[ki][:, mo:mo + mw],
                                         rhs=e2[ki][:],
                                         start=(ki == 0), stop=(ki == 1))
                    ge_ps.append(gep)

                # ES = E2 .* scores ; EG = E2 .* GE2
                es, eg = [], []
                for mi, (mo, mw) in enumerate(PCH):
                    t = p_e.tile([mw, W4], FP, tag=f"es{mi}")
                    nc.vector.tensor_tensor(t[:], e2[mi][:], s_ps[mi][:], OP.mult)
                    es.append(t)
                    t = p_e.tile([mw, W4], FP, tag=f"eg{mi}")
                    nc.vector.tensor_tensor(t[:], e2[mi][:], ge_ps[mi][:], OP.mult)
                    eg.append(t)

                # ---- num_u / wsq_u column sums via ones-matmul ----
                nw_ps = ps_nw.tile([W, 2 * NLOC], FP, tag="nw")
                for n in range(NLOC):
                    for j, src_t in ((0, es), (NLOC, eg)):
                        for ki, (ko, kw) in enumerate(PCH):
                            nc.tensor.matmul(
                                nw_ps[:, j + n:j + n + 1],
                                lhsT=src_t[ki][:, n * W:(n + 1) * W],
                                rhs=onescol[0:kw, :],
                                start=(ki == 0), stop=(ki == 1))

                # ---- finalize row_sim -> simexp[:, b, :] ----
                d2 = p_v.tile([W, NLOC], FP, tag="d2")
                nc.vector.tensor_tensor(d2[:], wnormsq[:], nw_ps[:, NLOC:2 * NLOC],
                                        OP.mult)
                dn = p_v.tile([W, NLOC], FP, tag="dn")
                nc.scalar.activation(dn[:], d2[:], AF.Sqrt)
                rdn = p_v.tile([W, NLOC], FP, tag="rdn")
                nc.vector.reciprocal(rdn[:], dn[:])
                rsn = p_v.tile([W, NLOC], FP, tag="rsn")
                nc.vector.tensor_tensor(rsn[:], nw_ps[:, 0:NLOC], rdn[:], OP.mult)
                nc.scalar.activation(se_view[:, b, :], rsn[:], AF.Exp,
                                     scale=temps_sb[0:W, 1:2])

                # ---- att_maps for the diagonal pair (iterations 0..3) ----
                if b < NLOC:
                    n = b
                    cs = ps_nw.tile([W, 1], FP, tag="nw")
                    for ki, (ko, kw) in enumerate(PCH):
                        nc.tensor.matmul(cs[:], lhsT=e2[ki][:, n * W:(n + 1) * W],
                                         rhs=onescol[0:kw, :],
                                         start=(ki == 0), stop=(ki == 1))
                    attT = ps_g.tile([W, S], FP, tag="g")
                    nc.tensor.transpose(attT[:, 0:128], e2[0][:, n * W:(n + 1) * W],
                                        eye_sb[:])
                    nc.tensor.transpose(attT[:, 128:S], e2[1][:, n * W:(n + 1) * W],
                                        eye_sb[0:68, 0:68])
                    rcq = p_v.tile([W, 1], FP, tag="rcq")
                    nc.vector.reciprocal(rcq[:], cs[:])
                    fac = p_v.tile([W, 1], FP, tag="fac")
                    nc.vector.tensor_tensor(fac[:], rcq[:], maskt_sb[:, n:n + 1],
                                            OP.mult)
                    atts = p_att.tile([W, S], FP, tag="atts")
                    nc.vector.tensor_scalar_mul(atts[:], attT[:], fac[:])
                    nc.sync.dma_start(d_att[:][n], atts[:])

        # ---- per-caption masked sum over q, log, store ----
        with nc.named_scope("tail"):
            lsum = ps_nw.tile([B, NLOC], FP, tag="nw")
            for n in range(NLOC):
                nc.tensor.matmul(lsum[:, n:n + 1], lhsT=se_view[:, :, n],
                                 rhs=maskt_sb[:, n:n + 1], start=True, stop=True)
            simlog = pers.tile([B, NLOC], FP)
            nc.scalar.activation(simlog[:], lsum[:], AF.Ln)
            nc.sync.dma_start(d_sims[:], simlog[:])

    nc.compile()
    return nc


def _get_nc():
    if "nc" not in _NC_CACHE:
        _NC_CACHE["nc"] = _build()
    return _NC_CACHE["nc"]


def kernel(img_features, words_emb, cap_lens, temp1, temp2, temp3):
    global LAST_RESULTS
    img = np.asarray(img_features, dtype=np.float32)
    wrd = np.asarray(words_emb, dtype=np.float32)
    lens = np.asarray(cap_lens, dtype=np.int32)
    t1, t2, t3 = float(temp1), float(temp2), float(temp3)

    eye = np.eye(128, dtype=np.float32)
    temps = np.tile(np.array([[t1, t2]], dtype=np.float32), (128, 1))
    mask_all = (np.arange(W)[None, :] < lens[:, None])          # [N, W] bool

    in_maps, orders = [], []
    for c in range(NCORE):
        caps = list(range(c * NLOC, (c + 1) * NLOC))
        order = caps + [b for b in range(B) if b not in caps]
        orders.append(order)
        m = mask_all[caps].astype(np.float32)                    # [4, 97]
        in_maps.append({
            "imgt": np.ascontiguousarray(img[order].transpose(0, 2, 1)),
            "words": np.ascontiguousarray(
                wrd[caps].transpose(1, 0, 2).reshape(C, W4)),
            "biasrow": (NEG * (1.0 - m)).reshape(1, W4),
            "wordt": np.ascontiguousarray(
                wrd[caps].transpose(2, 0, 1).reshape(W, NLOC * C)),
            "maskt": np.ascontiguousarray(m.T),
            "temps": temps,
            "eye": eye,
        })

    nc = _get_nc()
    res = run_bass_kernel_spmd(nc, in_maps, list(range(NCORE)),
                               trace=bool(os.environ.get("BASS_TRACE")))
    LAST_RESULTS = res

    sims = np.empty((B, B), dtype=np.float64)                    # [b, n]
    att = np.empty((B, W, S), dtype=np.float32)
    for c in range(NCORE):
        out_s = res.results[c]["out_sims"].astype(np.float64)    # [32, 4] permuted
        inv = np.empty(B, dtype=np.int64)
        inv[orders[c]] = np.arange(B)
        sims[:, c * NLOC:(c + 1) * NLOC] = out_s[inv]
        att[c * NLOC:(c + 1) * NLOC] = res.results[c]["out_att"]

    sims = sims * t3
    mx0 = sims.max(axis=1, keepdims=True)
    lse0 = np.log(np.exp(sims - mx0).sum(axis=1)) + mx0[:, 0]
    mx1 = sims.max(axis=0, keepdims=True)
    lse1 = np.log(np.exp(sims - mx1).sum(axis=0)) + mx1[0, :]
    diag = np.diagonal(sims)
    loss = ((lse0 - diag).mean() + (lse1 - diag).mean()) / 2.0
    return np.float32(loss), att
